# revision 1
# baseline (speedup 1.0000x reference)
"""TRN2 Bass/Tile kernel: causal self-attention with RoPE.

Sharding across 8 NeuronCores: batch (2) x head-groups (4 groups of 4 heads,
tensor-parallel). Each core computes, for its batch and its 4 heads:
Q/K/V projections (RoPE folded into doubled Q/K weight matmuls), causal
softmax attention in transposed (scores^T) orientation with the softmax
denominator obtained via an extra ones-column in V, and a partial output
projection. The host sums the 4 partial outputs per batch.

All matmuls run in float32r (TF32-like, full-rate for free dim >= 256,
fp32 PSUM accumulation); measured end-to-end rel error ~3e-4.
"""
import numpy as np
import ml_dtypes
import concourse.bass as bass
from concourse import bacc
import concourse.mybir as mybir
import concourse.tile as tile
from concourse.bass_utils import run_bass_kernel_spmd

B, S, D = 2, 2048, 1024
H, DK = 16, 64
THETA = 10000.0
ST = 512              # q-band / projection s-tile width
NSC = S // 128        # 16 s-chunks of 128
f32 = mybir.dt.float32
f32r = mybir.dt.float32r
bf16 = mybir.dt.bfloat16
AF = mybir.ActivationFunctionType
Alu = mybir.AluOpType

# v_aug layout per s-chunk, per head pair: A head [v(64) | one],
# B head [one | zeros(63) | v(64)] (places attn rows at psum partitions 64:128)
VA = 65
VB = 128
VHP = VA + VB        # 193
VSC = 2 * VHP        # 386

_NC = None
_CONSTS = None


def _build():
    import os
    phases = os.environ.get("K_PHASES", "ABC")
    nc = bacc.Bacc()
    xT = nc.dram_tensor("xT", [D, S], f32r, kind="ExternalInput")
    pw = nc.dram_tensor("pw", [D, 1280], f32r, kind="ExternalInput")
    woT = nc.dram_tensor("woT", [256, D], f32r, kind="ExternalInput")
    cossin = nc.dram_tensor("cossin", [128, 2 * S], f32, kind="ExternalInput")
    masks = nc.dram_tensor("masks", [128, 4096], bf16, kind="ExternalInput")
    vpat = nc.dram_tensor("vpat", [128, NSC * VSC], f32r, kind="ExternalInput")
    onesd = nc.dram_tensor("onesd", [128, 128], f32r, kind="ExternalInput")
    out = nc.dram_tensor("out", [S, D], f32, kind="ExternalOutput")

    with tile.TileContext(nc) as tc:
        with tc.tile_pool(name="persist", bufs=1) as pp:
            qT = [pp.tile([128, S], f32r, tag=f"qT{i}", name=f"qT{i}") for i in range(2)]
            kT = [pp.tile([128, S], f32r, tag=f"kT{i}", name=f"kT{i}") for i in range(2)]
            v_aug = pp.tile([128, NSC * VSC], f32r, tag="vaug")
            concatT = [pp.tile([128, S], f32r, tag=f"cT{i}", name=f"cT{i}") for i in range(2)]
            woT_sb = pp.tile([128, 2, D], f32r, tag="woT")
            ones_sb = pp.tile([128, 128], f32r, tag="ones")
            masks_sb = pp.tile([128, 4096], bf16, tag="masks")

            nc.sync.dma_start(masks_sb[:], masks[:])
            nc.sync.dma_start(ones_sb[:], onesd[:])
            nc.sync.dma_start(v_aug[:], vpat[:])
            nc.sync.dma_start(woT_sb[:],
                              woT[:].rearrange("(k p) m -> p k m", p=128))

            va_sc = v_aug[:].rearrange("p (c h r) -> p c h r", c=NSC, r=VHP)

            # ---- Phase A: projections + RoPE + V ----
            with tc.tile_pool(name="pa", bufs=1) as pa, \
                 tc.tile_pool(name="pax", bufs=2) as pax, \
                 tc.tile_pool(name="prope", bufs=4) as prope, \
                 tc.tile_pool(name="psA", bufs=6, space="PSUM") as psA, \
                 tc.tile_pool(name="psV", bufs=2, space="PSUM") as psV:
                pw_sb = pa.tile([128, 8, 1280], f32r, tag="pw")
                cs_sb = pa.tile([128, 2, S], f32, tag="cs")
                nc.sync.dma_start(pw_sb[:],
                                  pw[:].rearrange("(k p) m -> p k m", p=128))
                nc.sync.dma_start(cs_sb[:],
                                  cossin[:].rearrange("p (c s) -> p c s", c=2))

                for st in range(4):
                    xs = pax.tile([128, 8, ST], f32r, tag="xs")
                    nc.sync.dma_start(
                        xs[:],
                        xT[:, st * ST:(st + 1) * ST]
                        .rearrange("(k p) m -> p k m", p=128))
                    sl = slice(st * ST, (st + 1) * ST)
                    for hp in range(2):
                        for aoff, boff, dst in ((0, 256, qT), (512, 768, kT)):
                            pa_ps = psA.tile([128, ST], f32, tag="proj")
                            pb_ps = psA.tile([128, ST], f32, tag="proj")
                            ao = aoff + 128 * hp
                            bo = boff + 128 * hp
                            for kt in range(8):
                                nc.tensor.matmul(pa_ps[:],
                                                 pw_sb[:, kt, ao:ao + 128],
                                                 xs[:, kt, :],
                                                 start=(kt == 0), stop=(kt == 7))
                            for kt in range(8):
                                nc.tensor.matmul(pb_ps[:],
                                                 pw_sb[:, kt, bo:bo + 128],
                                                 xs[:, kt, :],
                                                 start=(kt == 0), stop=(kt == 7))
                            t1 = prope.tile([128, ST], f32r, tag="ropea")
                            t2 = prope.tile([128, ST], f32r, tag="ropeb")
                            nc.vector.tensor_tensor(t1[:], pa_ps[:],
                                                    cs_sb[:, 0, sl], Alu.mult)
                            nc.vector.tensor_tensor(t2[:], pb_ps[:],
                                                    cs_sb[:, 1, sl], Alu.mult)
                            nc.vector.tensor_tensor(dst[hp][:, sl], t1[:],
                                                    t2[:], Alu.add)
                    for scl in range(4):
                        sc = st * 4 + scl
                        vp = psV.tile([128, 256], f32, tag="vproj")
                        for kt in range(8):
                            nc.tensor.matmul(vp[:],
                                             xs[:, kt, scl * 128:(scl + 1) * 128],
                                             pw_sb[:, kt, 1024:1280],
                                             start=(kt == 0), stop=(kt == 7))
                        vp_r = vp[:].rearrange("p (g t e) -> p g t e", g=2, t=2)
                        nc.vector.tensor_copy(va_sc[:, sc, :, 0:64],
                                              vp_r[:, :, 0, :])
                        nc.vector.tensor_copy(va_sc[:, sc, :, VA + 64:VHP],
                                              vp_r[:, :, 1, :])

            # ---- Phase B: attention (scores^T -> exp -> PV -> normalize) ----
            if "B" not in phases:
                return _finish(nc)
            with tc.tile_pool(name="wtp", bufs=5) as wtp, \
                 tc.tile_pool(name="dnp", bufs=2) as dnp, \
                 tc.tile_pool(name="bcp", bufs=2) as bcp, \
                 tc.tile_pool(name="psS", bufs=2, space="PSUM") as psS, \
                 tc.tile_pool(name="psP", bufs=2, space="PSUM") as psP, \
                 tc.tile_pool(name="psB", bufs=1, space="PSUM") as psB:
                for band in range(4):
                    qsl = slice(band * ST, (band + 1) * ST)
                    nkt = 4 * band + 4
                    # diagonal k-tiles first: their mask multiply runs on
                    # gpsimd while PE/ACT stream the full (unmasked) k-tiles
                    kts = list(range(4 * band, nkt)) + list(range(0, 4 * band))
                    for hp in range(2):
                        pvA = psP.tile([65, ST], f32, tag="pv")
                        pvB = psP.tile([128, ST], f32, tag="pv")
                        for i, kt in enumerate(kts):
                            ksl = slice(kt * 128, (kt + 1) * 128)
                            scp = psS.tile([128, 1024], f32, tag="sc")
                            nc.tensor.matmul(scp[:, 0:512],
                                             kT[hp][0:64, ksl],
                                             qT[hp][0:64, qsl],
                                             start=True, stop=True)
                            nc.tensor.matmul(scp[:, 512:1024],
                                             kT[hp][64:128, ksl],
                                             qT[hp][64:128, qsl],
                                             start=True, stop=True)
                            wt = wtp.tile([128, 1024], f32r, tag="wt")
                            nc.scalar.activation(wt[:], scp[:], AF.Exp,
                                                 scale=0.125)
                            j = kt - 4 * band
                            if j >= 0:
                                eng = nc.vector if band == 0 else nc.gpsimd
                                eng.tensor_tensor(
                                    wt[:], wt[:],
                                    masks_sb[:, j * 1024:(j + 1) * 1024],
                                    Alu.mult)
                            nc.tensor.matmul(pvA[:],
                                             va_sc[:, kt, hp, 0:VA],
                                             wt[:, 0:512],
                                             start=(i == 0),
                                             stop=(i == nkt - 1),
                                             skip_group_check=True)
                            nc.tensor.matmul(pvB[:],
                                             va_sc[:, kt, hp, VA:VHP],
                                             wt[:, 512:1024],
                                             start=(i == 0),
                                             stop=(i == nkt - 1),
                                             skip_group_check=True)
                        # normalize head A (denominator at pvA row 64)
                        dnA = dnp.tile([65, ST], f32r, tag="dna")
                        nc.scalar.copy(dnA[64:65, :], pvA[64:65, :])
                        bcA_ps = psB.tile([64, ST], f32, tag="bca")
                        nc.tensor.matmul(bcA_ps[:], ones_sb[64:65, 0:64],
                                         dnA[64:65, :], start=True, stop=True)
                        bcA = bcp.tile([64, ST], f32, tag="bca")
                        nc.vector.reciprocal_approx_fast(bcA[:], bcA_ps[:])
                        nc.vector.tensor_tensor(concatT[hp][0:64, qsl],
                                                pvA[0:64, :], bcA[:], Alu.mult)
                        # normalize head B (denominator at pvB row 0,
                        # attn rows at 64:128)
                        rB = dnp.tile([65, ST], f32, tag="rb")
                        nc.vector.reciprocal_approx_fast(rB[0:1, :],
                                                         pvB[0:1, :])
                        dnB = dnp.tile([65, ST], f32r, tag="dnb")
                        nc.scalar.copy(dnB[0:1, :], rB[0:1, :])
                        bcB_ps = psB.tile([128, ST], f32, tag="bcb")
                        nc.tensor.matmul(bcB_ps[:], ones_sb[0:1, :],
                                         dnB[0:1, :], start=True, stop=True)
                        bcB = bcp.tile([128, ST], f32, tag="bcb")
                        nc.scalar.copy(bcB[64:128, :], bcB_ps[64:128, :])
                        nc.vector.tensor_tensor(concatT[hp][64:128, qsl],
                                                pvB[64:128, :], bcB[64:128, :],
                                                Alu.mult)

            # ---- Phase C: output projection (partial) ----
            if "C" not in phases:
                return _finish(nc)
            with tc.tile_pool(name="outp", bufs=3) as outp, \
                 tc.tile_pool(name="psO", bufs=2, space="PSUM") as psO:
                for sc in range(NSC):
                    ssl = slice(sc * 128, (sc + 1) * 128)
                    op_ps = psO.tile([128, D], f32, tag="op")
                    for ds in range(2):
                        dsl = slice(ds * 512, (ds + 1) * 512)
                        for ot in range(2):
                            nc.tensor.matmul(op_ps[:, dsl],
                                             concatT[ot][:, ssl],
                                             woT_sb[:, ot, dsl],
                                             start=(ot == 0), stop=(ot == 1))
                    ob = outp.tile([128, D], f32, tag="ob")
                    nc.vector.tensor_copy(ob[:], op_ps[:])
                    nc.sync.dma_start(out[ssl, :], ob[:])
    nc.finalize()
    return nc


def _rope_tables():
    inv_freq = 1.0 / (THETA ** (np.arange(0, DK, 2, dtype=np.float64) / DK))
    t = np.arange(S, dtype=np.float64)
    freqs = np.outer(t, inv_freq)
    emb = np.stack((freqs, freqs), axis=-1).reshape(S, DK)
    return np.cos(emb).astype(np.float32), np.sin(emb).astype(np.float32)


def _sgn_shuf(w):
    ws = np.empty_like(w)
    ws[0::2] = -w[1::2]
    ws[1::2] = w[0::2]
    return ws


def _host_consts():
    f_idx = np.arange(512)
    p_idx = np.arange(128)
    mblocks = []
    for j in range(4):
        mj = (f_idx[None, :] >= p_idx[:, None] + 128 * j).astype(np.float32)
        mblocks.append(np.tile(mj, (1, 2)))
    masks_np = np.concatenate(mblocks, axis=1).astype(ml_dtypes.bfloat16)

    vpat_np = np.zeros((128, NSC * VSC), np.float32)
    for sc in range(NSC):
        for r in range(2):
            base = sc * VSC + r * VHP
            vpat_np[:, base + 64] = 1.0   # A ones column
            vpat_np[:, base + VA] = 1.0   # B ones column

    onesd_np = np.zeros((128, 128), np.float32)
    onesd_np[64, 0:64] = 1.0              # lhsT for head-A broadcast
    onesd_np[0, 64:128] = 1.0             # lhsT for head-B broadcast
    return masks_np, vpat_np, onesd_np


def kernel(x, token_positions, W_q, W_k, W_v, W_o):
    global _NC
    if _NC is None:
        _NC = _build()
    x = np.asarray(x, dtype=np.float32)
    token_positions = np.asarray(token_positions)
    W_q = np.asarray(W_q, dtype=np.float32)
    W_k = np.asarray(W_k, dtype=np.float32)
    W_v = np.asarray(W_v, dtype=np.float32)
    W_o = np.asarray(W_o, dtype=np.float32)

    global _CONSTS
    if _CONSTS is None:
        _CONSTS = (*_rope_tables(), *_host_consts())
    cos_t, sin_t, masks_np, vpat_np, onesd_np = _CONSTS

    in_maps = []
    for c in range(8):
        b, g = divmod(c, 4)
        rows = slice(256 * g, 256 * (g + 1))
        wq, wk, wv = W_q[rows], W_k[rows], W_v[rows]
        pw_np = np.ascontiguousarray(np.concatenate(
            [wq.T, _sgn_shuf(wq).T, wk.T, _sgn_shuf(wk).T, wv.T], axis=1))
        woT_np = np.ascontiguousarray(W_o[:, rows].T)
        pos = np.asarray(token_positions[b], dtype=np.int64)
        cosT = np.tile(cos_t[pos].T, (2, 1))
        sinT = np.tile(sin_t[pos].T, (2, 1))
        cossin_np = np.ascontiguousarray(
            np.concatenate([cosT, sinT], axis=1), dtype=np.float32)
        xT_np = np.ascontiguousarray(x[b].T)
        in_maps.append({
            "xT": xT_np, "pw": pw_np, "woT": woT_np, "cossin": cossin_np,
            "masks": masks_np, "vpat": vpat_np, "onesd": onesd_np,
        })

    res = run_bass_kernel_spmd(_NC, in_maps, core_ids=list(range(8)))
    outs = [res.results[c]["out"] for c in range(8)]
    o0 = outs[0] + outs[1] + outs[2] + outs[3]
    o1 = outs[4] + outs[5] + outs[6] + outs[7]
    return np.stack([o0, o1]).astype(np.float32)



# revision 20
# speedup vs baseline: 2.1840x; 2.1840x over previous
"""TRN2 Bass/Tile kernel: causal self-attention with RoPE.

Sharding across 8 NeuronCores: batch (2) x head-groups (4 groups of 4 heads,
tensor parallel). Each core computes, for its batch and its 4 heads, the
Q/K/V projections, RoPE, causal softmax attention, and a partial output
projection; the host sums the 4 partial outputs per batch.

Key structure (chosen against the TimelineSim cost model, where a matmul
costs out-free-size x pe_cycle regardless of contraction/partition dims):
- All matmul operands bf16 (1 cyc/row; fp8 would blow the accuracy gate).
- RoPE via feature reorder [evens|odds] per head + partition-offset DMA
  swap of the projection tile, then cheap bf16 DVE mults (no doubled
  weight matmuls).
- Scores in [k-part, q-free] orientation; exp on ACT writes only the
  causally valid region; the 128x128 diagonal triangle is masked by one
  small DVE multiply; fully masked q-chunks are never computed or read.
- PV flipped: wt is the stationary operand, V ([s-part, feat]) moving, so
  attention comes out [q-part, feat] with the softmax denominator as a
  per-partition column (ones column in V) -> tensor_scalar normalize.
- PE transpose (bf16) re-orients normalized attention for the output
  projection.
- The issue order software-pipelines PE: within a band, scores(kt+1) is
  issued before PV(kt), and proj/outproj work is spread into the band's
  ACT-bound ktile stream in ~500ns matmul-level slices.
"""
import numpy as np
import ml_dtypes
import concourse.bass as bass
from concourse import bacc
import concourse.mybir as mybir
import concourse.tile as tile
from concourse.bass_utils import run_bass_kernel_spmd

B, S, D = 2, 2048, 1024
H, DK = 16, 64
THETA = 10000.0
ST = 512              # q-band / projection s-tile width
NSC = S // 128        # 16 s-chunks of 128
f32 = mybir.dt.float32
bf16 = mybir.dt.bfloat16
AF = mybir.ActivationFunctionType
Alu = mybir.AluOpType

_NC = None
_CONSTS = None


def _interleave(primary, front, even):
    """Issue primary units in order, pacing companion closures against the
    primaries' estimated PE-idle gaps. `front` closures are paced at 2x
    (finish ~halfway through); `even` closures spread across the whole
    stream. All lists hold (weight_ns, fn)."""
    ftot = sum(w for w, _ in front)
    etot = sum(w for w, _ in even)
    gap_total = sum(g for g, _ in primary) or 1.0
    gap_cum = fcum = ecum = 0.0
    fi = ei = 0
    for g, f in primary:
        f()
        gap_cum += g
        frac = gap_cum / gap_total
        while fi < len(front) and fcum < ftot * min(1.0, 2.0 * frac):
            w, cf = front[fi]
            cf()
            fcum += w
            fi += 1
        while ei < len(even) and ecum < etot * frac:
            w, cf = even[ei]
            cf()
            ecum += w
            ei += 1
    for _, cf in front[fi:]:
        cf()
    for _, cf in even[ei:]:
        cf()


def _build():
    import os
    phases = os.environ.get("K_PHASES", "ABC")
    debug = os.environ.get("K_DEBUG", "") == "1"
    nc = bacc.Bacc()
    xT = nc.dram_tensor("xT", [D, S], bf16, kind="ExternalInput")
    pw = nc.dram_tensor("pw", [D, 768], bf16, kind="ExternalInput")
    woT = nc.dram_tensor("woT", [256, D], bf16, kind="ExternalInput")
    cs = nc.dram_tensor("cs", [128, 2, S], bf16, kind="ExternalInput")
    tri = nc.dram_tensor("tri", [128, 256], bf16, kind="ExternalInput")
    ident = nc.dram_tensor("ident", [128, 128], bf16, kind="ExternalInput")
    out = nc.dram_tensor("out", [S, D], bf16, kind="ExternalOutput")
    if debug:
        dbg = {
            nm: nc.dram_tensor(nm, shp, bf16, kind="ExternalOutput")
            for nm, shp in (("d_qT0", [128, S]), ("d_qT1", [128, S]),
                            ("d_kT0", [128, S]), ("d_kT1", [128, S]),
                            ("d_cT0", [128, S]), ("d_cT1", [128, S]),
                            ("d_v", [128, NSC, 4, 65]),
                            ("d_wt0", [128, 1024]), ("d_wt1", [128, 1024]),
                            ("d_wt2", [128, 1024]), ("d_wt3", [128, 1024]),
                            ("d_pvA", [128, 260]), ("d_pvB", [128, 260]))
        }

    with tile.TileContext(nc) as tc:
        with tc.tile_pool(name="persist", bufs=1) as pp, \
             tc.tile_pool(name="pax", bufs=3) as pax, \
             tc.tile_pool(name="scpp", bufs=2, space="PSUM") as scpp, \
             tc.tile_pool(name="misc", bufs=2, space="PSUM") as misc, \
             tc.tile_pool(name="pspv", bufs=1, space="PSUM") as pspv, \
             tc.tile_pool(name="sswp", bufs=2) as sswp, \
             tc.tile_pool(name="st12", bufs=4) as st12, \
             tc.tile_pool(name="wtp", bufs=4) as wtp, \
             tc.tile_pool(name="anp", bufs=3) as anp, \
             tc.tile_pool(name="obp", bufs=3) as obp, \
             tc.tile_pool(name="nrm", bufs=4) as nrm:

            qT = [pp.tile([128, S], bf16, tag=f"qT{i}", name=f"qT{i}")
                  for i in range(2)]
            kT = [pp.tile([128, S], bf16, tag=f"kT{i}", name=f"kT{i}")
                  for i in range(2)]
            cT = [pp.tile([128, S], bf16, tag=f"cT{i}", name=f"cT{i}")
                  for i in range(2)]
            v_sb = pp.tile([128, NSC, 4, 65], bf16, tag="vsb")
            woT_sb = pp.tile([128, 2, D], bf16, tag="woT")
            cs_sb = pp.tile([128, 2, S], bf16, tag="cs")
            pw_sb = pp.tile([128, 8, 768], bf16, tag="pw")
            tri_sb = pp.tile([128, 256], bf16, tag="tri")
            id_sb = pp.tile([128, 128], bf16, tag="id")

            pw4 = pw[:].rearrange("(k p) m -> p k m", p=128)
            tri3 = tri_sb[:].rearrange("p (h q) -> p h q", h=2)
            xs_tiles = {}

            def u_xload(st):
                def f():
                    xs = pax.tile([128, 8, ST], bf16, tag="xs", name=f"xs{st}")
                    x4 = xT[:, st * ST:(st + 1) * ST].rearrange(
                        "(k p) m -> p k m", p=128)
                    nc.sync.dma_start(xs[:, 0:4, :], x4[:, 0:4, :])
                    nc.sync.dma_start(xs[:, 4:8, :], x4[:, 4:8, :])
                    xs_tiles[st] = xs
                return (0, f)

            # pw column layout: [qk-hp0 (q128|k128) | qk-hp1 | v 256]
            def projqk_closures(st, hp):
                sl = slice(st * ST, (st + 1) * ST)
                state = {}
                cls = []

                def mk_mms(half, base, lo):
                    def f():
                        if "qk_sb" not in state:
                            state["qk_sb"] = sswp.tile([128, 1024], bf16,
                                                       tag="qk", name="qksb")
                        if half not in state:
                            state[half] = misc.tile([128, 512], f32, tag="m1",
                                                    name="qkps")
                        h_ps = state[half]
                        xs = xs_tiles[st]
                        for kt in range(lo, lo + 2):
                            nc.tensor.matmul(
                                h_ps[:],
                                pw_sb[:, kt, base:base + 128],
                                xs[:, kt, :],
                                start=(kt == 0), stop=(kt == 7))
                    return f

                def mk_copy(half):
                    def f():
                        nc.vector.tensor_copy(
                            state["qk_sb"][:, half * 512:(half + 1) * 512],
                            state[half][:])
                    return f

                def fin():
                    qk_sb = state["qk_sb"]
                    qksw = sswp.tile([128, 1024], bf16, tag="sw", name="qksw")
                    for lo, hi in ((0, 32), (32, 0), (64, 96), (96, 64)):
                        nc.sync.dma_start(qksw[lo:lo + 32, :],
                                          qk_sb[hi:hi + 32, :])
                    for half, dst in ((0, qT), (1, kT)):
                        hs = slice(half * 512, (half + 1) * 512)
                        t1 = st12.tile([128, ST], bf16, tag="t1", name="t1")
                        t2 = st12.tile([128, ST], bf16, tag="t2", name="t2")
                        nc.vector.tensor_tensor(t1[:], qk_sb[:, hs],
                                                cs_sb[:, 0, sl], Alu.mult)
                        nc.vector.tensor_tensor(t2[:], qksw[:, hs],
                                                cs_sb[:, 1, sl], Alu.mult)
                        nc.vector.tensor_tensor(dst[hp][:, sl], t1[:],
                                                t2[:], Alu.add)

                for half in range(2):
                    base = 256 * hp + 128 * half
                    for lo in range(0, 8, 2):
                        cls.append((427, mk_mms(half, base, lo)))
                    cls.append((0, mk_copy(half)))
                cls.append((0, fin))
                return cls

            def projv_closures(st, scl):
                sc = st * 4 + scl
                state = {}

                def mk_mms(lo):
                    def f():
                        if "vp" not in state:
                            state["vp"] = misc.tile([128, 512], f32, tag="m1",
                                                    name="vp")
                        vp = state["vp"]
                        xs = xs_tiles[st]
                        for kt in range(lo, lo + 4):
                            nc.tensor.matmul(
                                vp[:, 0:256],
                                xs[:, kt, scl * 128:(scl + 1) * 128],
                                pw_sb[:, kt, 512:768],
                                start=(kt == 0), stop=(kt == 7))
                    return f

                def fin():
                    nc.vector.tensor_copy(
                        v_sb[:, sc, :, 0:64],
                        state["vp"][:, 0:256].rearrange("p (h f) -> p h f",
                                                        h=4))
                return [(427, mk_mms(0)), (427, mk_mms(4)), (0, fin)]

            def dma_closure(*pairs):
                def f():
                    for dst, src in pairs:
                        nc.sync.dma_start(dst, src)
                return (0, f)

            def proj_closures(st):
                cls = projqk_closures(st, 0)
                cls += projqk_closures(st, 1)
                for scl in range(4):
                    cls += projv_closures(st, scl)
                return cls

            def outproj_closures(b, per_chunk=False, act_copy=False):
                chunks = []
                for sc in range(b * 4, b * 4 + 4):
                    ssl = slice(sc * 128, (sc + 1) * 128)
                    state = {}

                    def mk_mms(sc, ssl, state, ds):
                        dsl = slice(ds * 512, (ds + 1) * 512)

                        def f():
                            if "ob" not in state:
                                state["ob"] = obp.tile([128, D], bf16,
                                                       tag="ob", name="ob")
                            op_ps = misc.tile([128, 512], f32, tag="m1",
                                              name="opps")
                            state[ds] = op_ps
                            for hp in range(2):
                                nc.tensor.matmul(op_ps[:],
                                                 cT[hp][:, ssl],
                                                 woT_sb[:, hp, dsl],
                                                 start=(hp == 0),
                                                 stop=(hp == 1))
                        return f

                    def mk_fin(sc, ssl, state, ds, last):
                        dsl = slice(ds * 512, (ds + 1) * 512)

                        def f():
                            if act_copy:
                                nc.scalar.copy(state["ob"][:, dsl],
                                               state[ds][:])
                            else:
                                nc.vector.tensor_copy(state["ob"][:, dsl],
                                                      state[ds][:])
                            if last:
                                nc.sync.dma_start(out[ssl, :], state["ob"][:])
                        return f

                    chunks.append([
                        (427, mk_mms(sc, ssl, state, 0)),
                        (0, mk_fin(sc, ssl, state, 0, False)),
                        (427, mk_mms(sc, ssl, state, 1)),
                        (0, mk_fin(sc, ssl, state, 1, True)),
                    ])
                if per_chunk:
                    return chunks
                return [c for ch in chunks for c in ch]

            def u_ktile(b, hp, kt, pvs):
                def f():
                    nkt = 4 * b + 4
                    j = kt - 4 * b
                    # exp + tri for kt (scores already issued)
                    off = 128 * j if j > 0 else 0
                    scp3 = pvs["scp"][kt][:].rearrange("p (h q) -> p h q", h=2)
                    wt = wtp.tile([128, 1024], bf16, tag="wt", name="wt")
                    wt3 = wt[:].rearrange("p (h q) -> p h q", h=2)
                    if debug and b == 0 and hp == 0:
                        nc.vector.memset(wt[:], 0.0)
                    nc.scalar.activation(wt3[:, :, off:512],
                                         scp3[:, :, off:512],
                                         AF.Exp, scale=0.125)
                    if j >= 0:
                        nc.vector.tensor_tensor(
                            wt3[:, :, 128 * j:128 * j + 128],
                            wt3[:, :, 128 * j:128 * j + 128],
                            tri3[:], Alu.mult)
                    if debug and b == 0 and hp == 0:
                        nc.sync.dma_start(dbg[f"d_wt{kt}"][:], wt[:])
                    # scores for kt+1 ahead of PV(kt)
                    if kt + 1 < nkt:
                        issue_scores(b, hp, kt + 1, pvs)
                    for h in range(2):
                        for qc in range(max(0, j), 4):
                            # start=True marks the WHOLE 2KB psum bank
                            # pending-zero, so only the first matmul into
                            # each pv bank per band may carry it; later
                            # groups' first writes are zeroed by the same
                            # bank-wide flag.
                            nc.tensor.matmul(
                                pvs["pv"][h][:, 65 * qc:65 * qc + 65],
                                wt3[:, h, 128 * qc:128 * qc + 128],
                                v_sb[:, kt, 2 * hp + h, :],
                                start=(kt == 0 and qc == 0),
                                stop=(kt == 4 * b + qc),
                                skip_group_check=True)
                return f

            def issue_scores(b, hp, kt, pvs):
                j = kt - 4 * b
                off = 128 * j if j > 0 else 0
                ktsl = slice(kt * 128, (kt + 1) * 128)
                scp = scpp.tile([128, 1024], f32, tag="scp", name="scp")
                pvs["scp"][kt] = scp
                scp3 = scp[:].rearrange("p (h q) -> p h q", h=2)
                for h in range(2):
                    nc.tensor.matmul(
                        scp3[:, h, off:512],
                        kT[hp][64 * h:64 * h + 64, ktsl],
                        qT[hp][64 * h:64 * h + 64,
                               b * ST + off:(b + 1) * ST],
                        start=True, stop=True)

            def u_norm_dve(b, hp, pvs, state):
                def f():
                    if debug and b == 0 and hp == 0:
                        for h, nm in ((0, "d_pvA"), (1, "d_pvB")):
                            dcp = nrm.tile([128, 260], bf16, tag="dcp",
                                           name="dcp")
                            nc.vector.tensor_copy(dcp[:], pvs["pv"][h][:])
                            nc.sync.dma_start(dbg[nm][:], dcp[:])
                    rcs = []
                    for h in range(2):
                        rc = nrm.tile([128, 4], f32, tag="rc", name="rc")
                        nc.vector.reciprocal_approx_fast(
                            rc[:],
                            pvs["pv"][h][:].rearrange("p (qc e) -> p qc e",
                                                      e=65)[:, :, 64])
                        rcs.append(rc)
                    ans = []
                    for qc in range(4):
                        an = anp.tile([128, 128], bf16, tag="an", name="an")
                        for h in range(2):
                            nc.vector.tensor_scalar(
                                an[:, 64 * h:64 * h + 64],
                                pvs["pv"][h][:, 65 * qc:65 * qc + 64],
                                rcs[h][:, qc:qc + 1], None, Alu.mult)
                        ans.append(an)
                    state["ans"] = ans
                return f

            def u_norm_pe(b, hp, state):
                def f():
                    tp = misc.tile([128, 1024], bf16, tag="m1", name="tp")
                    for qc in range(4):
                        nc.tensor.transpose(
                            tp[:, qc * 128:(qc + 1) * 128],
                            state["ans"][qc], id_sb[:])
                    nc.vector.tensor_copy(
                        cT[hp][:, b * ST:(b + 1) * ST], tp[:, 0:512])
                return f

            def u_norm_qc(b, hp, pvs, qc):
                # per-q-chunk normalize + transpose + cT copy (band tail)
                def f():
                    rcs = []
                    for h in range(2):
                        rc = nrm.tile([128, 4], f32, tag="rc", name="rc")
                        nc.vector.reciprocal_approx_fast(
                            rc[:, 0:1],
                            pvs["pv"][h][:, 65 * qc + 64:65 * qc + 65])
                        rcs.append(rc)
                    an = anp.tile([128, 128], bf16, tag="an", name="an")
                    for h in range(2):
                        nc.vector.tensor_scalar(
                            an[:, 64 * h:64 * h + 64],
                            pvs["pv"][h][:, 65 * qc:65 * qc + 64],
                            rcs[h][:, 0:1], None, Alu.mult)
                    tp = misc.tile([128, 512], f32, tag="m1", name="tpq")
                    tpb = tp[:].bitcast(bf16)
                    nc.tensor.transpose(tpb[:, 0:128], an[:], id_sb[:])
                    nc.vector.tensor_copy(
                        cT[hp][:, b * ST + qc * 128:b * ST + qc * 128 + 128],
                        tpb[:, 0:128])
                return f

            def band_units(b, tail_cls=()):
                # returns [(gap_ns, fn)] with per-unit PE-idle estimates
                us = []
                nkt = 4 * b + 4
                for hp in range(2):
                    pvA = pspv.tile([128, 260], f32, tag="pvA",
                                    name=f"pvA{b}{hp}")
                    pvB = pspv.tile([128, 260], f32, tag="pvB",
                                    name=f"pvB{b}{hp}")
                    pvs = {"pv": (pvA, pvB), "scp": {}}

                    def mk_start(b, hp, pvs):
                        def f():
                            issue_scores(b, hp, 0, pvs)
                        return f
                    us.append((200 if hp == 0 else 0, mk_start(b, hp, pvs)))
                    for kt in range(nkt):
                        j = kt - 4 * b
                        off = 128 * j if j > 0 else 0
                        act = (1024 - 2 * off) * 0.833 + 185
                        nxt = kt + 1
                        pe = 0.0
                        if nxt < nkt:
                            offn = 128 * (nxt - 4 * b) if nxt > 4 * b else 0
                            pe += (1024 - 2 * offn) * 0.4167
                        pe += (8 - 2 * max(0, j)) * 27.1
                        us.append((max(0.0, act - pe),
                                   u_ktile(b, hp, kt, pvs)))
                    if b == 3 and hp == 1 and tail_cls:
                        # pipelined tail: per-qc norm + outproj chunks
                        for qc in range(4):
                            us.append((600, u_norm_qc(b, hp, pvs, qc)))
                            for wt_, fn_ in tail_cls[qc]:
                                us.append((0, fn_))
                    else:
                        state = {}
                        us.append((1400, u_norm_dve(b, hp, pvs, state)))
                        us.append((200, u_norm_pe(b, hp, state)))
                return us

            # ---- pipeline schedule ----
            xs0 = pax.tile([128, 8, ST], bf16, tag="xs", name="xs0")
            x40 = xT[:, 0:ST].rearrange("(k p) m -> p k m", p=128)
            nc.sync.dma_start(pw_sb[:, 0:2, 0:128], pw4[:, 0:2, 0:128])
            nc.sync.dma_start(xs0[:, 0:2, :], x40[:, 0:2, :])
            nc.sync.dma_start(pw_sb[:, 2:8, 0:128], pw4[:, 2:8, 0:128])
            nc.sync.dma_start(xs0[:, 2:4, :], x40[:, 2:4, :])
            nc.sync.dma_start(pw_sb[:, :, 128:256], pw4[:, :, 128:256])
            nc.sync.dma_start(xs0[:, 4:8, :], x40[:, 4:8, :])
            xs_tiles[0] = xs0
            nc.sync.dma_start(cs_sb[:], cs[:])
            nc.sync.dma_start(pw_sb[:, :, 256:512], pw4[:, :, 256:512])
            nc.sync.dma_start(tri_sb[:], tri[:])
            nc.sync.dma_start(id_sb[:], ident[:])
            nc.vector.memset(v_sb[:, :, :, 64], 1.0)
            for _w, f in projqk_closures(0, 0) + projqk_closures(0, 1):
                f()
            nc.sync.dma_start(pw_sb[:, :, 512:768], pw4[:, :, 512:768])
            for scl in range(4):
                for _w, f in projv_closures(0, scl):
                    f()
            if "B" in phases:
                for b in range(4):
                    if b < 3:
                        u_xload(b + 1)[1]()
                    front = []
                    even = []
                    if b < 3:
                        front += projqk_closures(b + 1, 0)
                        front += projqk_closures(b + 1, 1)
                        for scl in range(4):
                            even += projv_closures(b + 1, scl)
                    if b == 2:
                        even += [dma_closure(
                            (woT_sb[:],
                             woT[:].rearrange("(k p) m -> p k m", p=128)))]
                    tail = ()
                    if b == 3 and "C" in phases:
                        even += outproj_closures(0)
                        even += outproj_closures(1)
                        even += outproj_closures(2)
                        tail = outproj_closures(3, per_chunk=True, act_copy=True)
                    _interleave(band_units(b, tail), front, even)
            if debug:
                for nm, t in (("d_qT0", qT[0]), ("d_qT1", qT[1]),
                              ("d_kT0", kT[0]), ("d_kT1", kT[1]),
                              ("d_cT0", cT[0]), ("d_cT1", cT[1]),
                              ("d_v", v_sb)):
                    nc.sync.dma_start(dbg[nm][:], t[:])
    nc.finalize()
    return nc


def _host_consts():
    # RoPE tables in [evens|odds] per-32-block row layout, sign folded into
    # the sin table. Row p: freq index p%32; blocks 0,2 (even slots) carry
    # -sin, blocks 1,3 (odd slots) +sin.
    inv_freq = 1.0 / (THETA ** (np.arange(0, DK, 2, dtype=np.float64) / DK))

    tri_np = np.zeros((128, 256), np.float32)
    p = np.arange(128)
    q = np.arange(128)
    blk = (q[None, :] >= p[:, None]).astype(np.float32)
    tri_np[:, 0:128] = blk
    tri_np[:, 128:256] = blk
    id_np = np.eye(128, dtype=np.float32)
    return (inv_freq,
            tri_np.astype(ml_dtypes.bfloat16),
            id_np.astype(ml_dtypes.bfloat16))


def _cs_table(pos, inv_freq):
    # pos: [S] int positions for this batch -> cs [128, 2, S] bfloat16
    i = np.arange(128) % 32
    ang = pos[None, :].astype(np.float64) * inv_freq[i][:, None]  # [128, S]
    cs = np.empty((128, 2, len(pos)), np.float32)
    cs[:, 0, :] = np.cos(ang)
    sgn = np.where(((np.arange(128) // 32) % 2) == 0, -1.0, 1.0)
    cs[:, 1, :] = sgn[:, None] * np.sin(ang)
    return cs.astype(ml_dtypes.bfloat16)


_EO_PERM = None


def _eo_perm():
    # per-head [evens | odds] feature permutation for 256 q/k rows (4 heads)
    global _EO_PERM
    if _EO_PERM is None:
        perm = []
        for h in range(4):
            perm.extend(64 * h + np.arange(0, 64, 2))
            perm.extend(64 * h + np.arange(1, 64, 2))
        _EO_PERM = np.asarray(perm)
    return _EO_PERM


def kernel(x, token_positions, W_q, W_k, W_v, W_o):
    global _NC, _CONSTS
    if _NC is None:
        _NC = _build()
    if _CONSTS is None:
        _CONSTS = _host_consts()
    inv_freq, tri_np, id_np = _CONSTS

    x = np.asarray(x, dtype=np.float32)
    token_positions = np.asarray(token_positions)
    W_q = np.asarray(W_q, dtype=np.float32)
    W_k = np.asarray(W_k, dtype=np.float32)
    W_v = np.asarray(W_v, dtype=np.float32)
    W_o = np.asarray(W_o, dtype=np.float32)

    perm = _eo_perm()
    cs_by_batch = [
        _cs_table(np.asarray(token_positions[b], dtype=np.int64), inv_freq)
        for b in range(B)
    ]
    xT_by_batch = [
        np.ascontiguousarray(x[b].T).astype(ml_dtypes.bfloat16)
        for b in range(B)
    ]

    in_maps = []
    for c in range(8):
        b, g = divmod(c, 4)
        rows = slice(256 * g, 256 * (g + 1))
        wq = W_q[rows][perm]
        wk = W_k[rows][perm]
        wv = W_v[rows]
        # pw columns: [q-hp0 | k-hp0 | q-hp1 | k-hp1 | v]
        pw_np = np.ascontiguousarray(np.concatenate(
            [wq[0:128].T, wk[0:128].T, wq[128:256].T, wk[128:256].T, wv.T],
            axis=1)).astype(ml_dtypes.bfloat16)
        woT_np = np.ascontiguousarray(W_o[:, rows].T).astype(
            ml_dtypes.bfloat16)
        in_maps.append({
            "xT": xT_by_batch[b], "pw": pw_np, "woT": woT_np,
            "cs": cs_by_batch[b], "tri": tri_np, "ident": id_np,
        })

    res = run_bass_kernel_spmd(_NC, in_maps, core_ids=list(range(8)))
    outs = [np.asarray(res.results[c]["out"], np.float32) for c in range(8)]
    o0 = outs[0] + outs[1] + outs[2] + outs[3]
    o1 = outs[4] + outs[5] + outs[6] + outs[7]
    return np.stack([o0, o1]).astype(np.float32)
